# revision 18
# baseline (speedup 1.0000x reference)
"""Trainium2 Bass kernel for the HPLSTM module (8-core SPMD, sequence-parallel).

Math (per reference):
    fg = sigmoid(x @ Wf + bf)
    hr = sigmoid(x @ Wi + bi) * tanh(x @ Wh + bh)
    c_t = fg_t * c_{t-1} + hr_t              (linear scan over time)
    og = sigmoid([x, c] @ Wo + bo)
    o  = (og * c) @ Wout + bout

Sharding: sequence-parallel. Core k owns timesteps [k*1024, (k+1)*1024) and
additionally recomputes a WARM-step prefix to derive its scan initial
condition locally (forget gates are sigmoid(~N(0,1)), so carry contributions
decay like ~0.45^t and are far below fp32 resolution after WARM steps). No
cross-core communication at all.

Layout: activations live transposed as [hidden, time] so the recurrence runs
along the SBUF free axis via the DVE tensor_tensor_scan instruction. The
final projection consumes (og*c) in [hidden, time] layout directly as the
matmul stationary operand and produces output in natural [time, d_out]
orientation, so no transpose-back is needed.

Precision: the i/h gate GEMMs and the output projection run in fp16 (fp32
PSUM accumulation). The forget-gate GEMM and the output-gate GEMM run in
fp8-e4m3 with perf_mode=DoubleRow (2 fp8 weights/PE cell, 256-deep
contraction per matmul, ~2x PE throughput) — these are the two spots where
fp8 quantization noise barely reaches the output (sigmoid gain 0.25; the
scan and the og path attenuate further). Wf/Wo are pre-scaled by 64
host-side so their entries sit in e4m3's normal range; the sigmoid
activations un-scale via their scale argument.

Loop order: contraction-outer, n-chunk-inner over multiple live PSUM banks,
so every LDWEIGHTS hides under a preceding 512-wide matmul (a 64-wide
matmul can't cover a weight load on its own).

DMA: all HBM loads use >=2KB per-partition lines (the per-queue descriptor
feed is ~78ns/line — smaller lines are feed-bound) and big tensors are
split across several dma_starts so the round-robin queue assignment spreads
them over the 16 HW queues.
"""

import numpy as np

import concourse.bacc as bacc
import concourse.mybir as mybir
import concourse.tile as tile
from concourse.bass_utils import run_bass_kernel_spmd

SEQ, D_IN, D_HID, D_OUT = 8192, 2048, 2048, 2048
N_CORES = 8
P = 128
S_OWN = SEQ // N_CORES          # 1024 timesteps owned per core
WARM = 64                       # truncated-carry warmup prefix
S_TOT = S_OWN + WARM            # 1088 time columns held per core
KC = D_IN // P                  # 16 contraction chunks over d_in
MC = D_HID // P                 # 16 chunks over hidden
KQ = KC // 2                    # 8 DoubleRow chunks (256-deep) over d_in
NF = 512                        # PSUM moving free-dim

MM_DT = mybir.dt.float16        # fp16 matmul operand dtype (fp32 PSUM accum)
MM_NP = np.float16
F8 = mybir.dt.float8e4          # TRN e4m3 (max 240)
WSCALE = 64.0                   # host-side scale on Wf/Wo before fp8 cast

F32 = mybir.dt.float32

_BUILD_CACHE = {}


def build_module():
    """Build + compile the single-core BIR module (same NEFF on all 8 cores)."""
    act = mybir.ActivationFunctionType
    alu = mybir.AluOpType
    DR = mybir.MatmulPerfMode.DoubleRow

    nc = bacc.Bacc("TRN2", debug=False, num_devices=N_CORES)

    xT = nc.declare_dram_parameter("xT", [D_IN, S_TOT], MM_DT, isOutput=False)
    xT8 = nc.declare_dram_parameter("xT8", [D_IN, S_TOT], F8, isOutput=False)
    # i/h gate weights pre-tiled host-side: [2, MC, P, KC*P] (4KB lines)
    wg = nc.declare_dram_parameter("Wg", [2, MC, P, KC * P], MM_DT, isOutput=False)
    # forget-gate weights, fp8 DoubleRow-interleaved, pre-scaled by WSCALE:
    # [MC, P, KQ*2*P] with k = 256*q + 128*j + p
    wf8 = nc.declare_dram_parameter("Wf8", [MC, P, KQ * 2 * P], F8, isOutput=False)
    # output-gate weights (x-part, c-part), same fp8 layout
    wo = nc.declare_dram_parameter("Wo8", [2, MC, P, KQ * 2 * P], F8, isOutput=False)
    wout = nc.declare_dram_parameter("Wout", [MC, P, D_OUT], MM_DT, isOutput=False)
    bg = nc.declare_dram_parameter("bg", [P, 3, MC], F32, isOutput=False)
    bo = nc.declare_dram_parameter("bo", [P, MC], F32, isOutput=False)
    bout = nc.declare_dram_parameter("bout", [P, D_OUT], F32, isOutput=False)
    out = nc.declare_dram_parameter("out", [S_OWN, D_OUT], F32, isOutput=True)

    # n-chunks over the full (warm + owned) time range for the gate GEMMs
    NCH = [(0, NF), (NF, NF), (2 * NF, S_TOT - 2 * NF)]

    with tile.TileContext(nc) as tc:
        with (
            tc.tile_pool(name="singles", bufs=1) as singles,
            tc.tile_pool(name="wpool", bufs=4) as wpool,
            tc.tile_pool(name="w8pool", bufs=4) as w8pool,
            tc.tile_pool(name="wcpool", bufs=20) as wcpool,
            tc.tile_pool(name="gpool", bufs=2) as gpool,
            tc.tile_pool(name="spool", bufs=2) as spool,
            tc.tile_pool(name="psum", bufs=6, space="PSUM") as pspool,
        ):
            xT_sb = singles.tile([P, KC, S_TOT], MM_DT)
            xT8_sb = singles.tile([P, KC, S_TOT], F8)
            c_sb = singles.tile([P, MC, S_TOT], MM_DT)
            c8_sb = singles.tile([P, MC, S_OWN], F8)
            mog_sb = singles.tile([P, MC, S_OWN], MM_DT)
            bg_sb = singles.tile([P, 3, MC], F32)
            bo_sb = singles.tile([P, MC], F32)
            bout_sb = singles.tile([P, D_OUT], F32)

            def load_wa(g, mc):
                """i/h fp16 weight tile [P, KC*P]; 2KB lines, 2 calls."""
                wt = wpool.tile([P, KC * P], MM_DT, tag="w")
                src = wg.ap()[g, mc]
                for v in range(2):
                    nc.sync.dma_start(
                        out=wt[:, 1024 * v : 1024 * (v + 1)],
                        in_=src[:, 1024 * v : 1024 * (v + 1)],
                    )
                return wt

            def load_w8(src_ap, eng=None):
                """fp8 DoubleRow weight tile [P, KQ, 2, P]; 2KB lines, 2 calls."""
                wt = w8pool.tile([P, KQ, 2, P], F8, tag="w8")
                src = src_ap.rearrange("p (q j m) -> p q j m", q=KQ, j=2)
                e = eng if eng is not None else nc.sync
                e.dma_start(out=wt[:64], in_=src[:64])
                e.dma_start(out=wt[64:], in_=src[64:])
                return wt

            # ---- prologue DMAs. Each dma_start costs ~645ns of serial
            # dispatch on its engine's queue, so the critical path is call
            # COUNT on the sync queue: xT16 rows start dispatching
            # immediately (interleaved with the mc=0 weight calls), while
            # the fp8 operands (needed ~25us in) dispatch in parallel from
            # the otherwise-idle gpsimd queue and the biases from scalar.
            xT_t = xT.ap().rearrange("(kc p) t -> kc p t", p=P)
            xT8_t = xT8.ap().rearrange("(kc p) t -> kc p t", p=P)
            for kc in range(2):
                nc.sync.dma_start(out=xT_sb[:, kc], in_=xT_t[kc])
            wi0 = load_wa(0, 0)
            for kc in range(2, 8):
                nc.sync.dma_start(out=xT_sb[:, kc], in_=xT_t[kc])
            wh0 = load_wa(1, 0)
            for kc in range(8, KC):
                nc.sync.dma_start(out=xT_sb[:, kc], in_=xT_t[kc])
            wih_next = [wi0, wh0]
            xT8_pk = xT8.ap().rearrange("(kc p) t -> p kc t", p=P)
            for kc in range(0, KC, 2):
                nc.gpsimd.dma_start(
                    out=xT8_sb[:, kc : kc + 2], in_=xT8_pk[:, kc : kc + 2]
                )
            wf_next = load_w8(wf8.ap()[0], eng=nc.gpsimd)
            nc.scalar.dma_start(out=bg_sb, in_=bg.ap())
            nc.scalar.dma_start(out=bo_sb, in_=bo.ap())
            nc.scalar.dma_start(out=bout_sb, in_=bout.ap())

            # ---- Stage A: gate GEMMs + activations + scan, per hidden chunk
            for mc in range(MC):
                # depth-1 weight prefetch: issue mc+1's loads before mc's
                # compute so their DMAs dispatch a full gate-group early
                wih_cur, wf_cur = wih_next, wf_next
                if mc + 1 < MC:
                    wih_next = [load_wa(g, mc + 1) for g in range(2)]
                    wf_next = load_w8(wf8.ap()[mc + 1])

                # i (g=1) and h (g=2) gates in fp16, kc-outer over 3 banks
                ih_tiles = []
                for g in (1, 2):
                    wt = wih_cur[g - 1]
                    g_sb = gpool.tile([P, S_TOT], MM_DT, tag=f"g{g}")
                    pss = [pspool.tile([P, nw], F32, tag="ps", name=f"ps{j}") for j, (_, nw) in enumerate(NCH)]
                    for kc in range(KC):
                        for j, (n0, nw) in enumerate(NCH):
                            nc.tensor.matmul(
                                out=pss[j],
                                lhsT=wt[:, P * kc : P * (kc + 1)],
                                rhs=xT_sb[:, kc, n0 : n0 + nw],
                                start=(kc == 0),
                                stop=(kc == KC - 1),
                            )
                    fn = act.Tanh if g == 2 else act.Sigmoid
                    for j, (n0, nw) in enumerate(NCH):
                        nc.scalar.activation(
                            out=g_sb[:, n0 : n0 + nw],
                            in_=pss[j],
                            func=fn,
                            bias=bg_sb[:, g, mc : mc + 1],
                        )
                    ih_tiles.append(g_sb)

                # forget gate in fp8 DoubleRow, kq-outer over 3 banks
                wtf = wf_cur
                f_sb = gpool.tile([P, S_TOT], MM_DT, tag="g0")
                pss = [pspool.tile([P, nw], F32, tag="ps", name=f"ps{j}") for j, (_, nw) in enumerate(NCH)]
                for q in range(KQ):
                    for j, (n0, nw) in enumerate(NCH):
                        nc.tensor.matmul(
                            out=pss[j],
                            lhsT=wtf[:, q],
                            rhs=xT8_sb[:, 2 * q : 2 * q + 2, n0 : n0 + nw],
                            start=(q == 0),
                            stop=(q == KQ - 1),
                            perf_mode=DR,
                        )
                for j, (n0, nw) in enumerate(NCH):
                    nc.scalar.activation(
                        out=f_sb[:, n0 : n0 + nw],
                        in_=pss[j],
                        func=act.Sigmoid,
                        bias=bg_sb[:, 0, mc : mc + 1],
                        scale=1.0 / WSCALE,
                    )

                # hr = sig(i)*tanh(h), in-place over the tanh tile
                nc.vector.tensor_mul(out=ih_tiles[1], in0=ih_tiles[0], in1=ih_tiles[1])
                # c_t = fg_t * c_{t-1} + hr_t along the free (time) axis
                nc.vector.tensor_tensor_scan(
                    out=c_sb[:, mc, :],
                    data0=f_sb,
                    data1=ih_tiles[1],
                    initial=0.0,
                    op0=alu.mult,
                    op1=alu.add,
                )
                # fp8 copy of the owned region for the stage-B moving operand
                # (last chunks go on the faster scalar engine — they gate
                # stage B's first PSUM group at the A->B boundary)
                if mc >= 14:
                    nc.scalar.copy(out=c8_sb[:, mc], in_=c_sb[:, mc, WARM:])
                else:
                    nc.gpsimd.tensor_copy(out=c8_sb[:, mc], in_=c_sb[:, mc, WARM:])

            # ---- Stage B: output gate over [x; c] in fp8 DoubleRow,
            # then mog = og * c
            wv0 = []
            for mc in range(MC):
                wts = [load_w8(wo.ap()[part, mc]) for part in range(2)]
                # prefetch stage C's first weight block in waves on the
                # scalar queue so it never delays stage B's own loads
                if mc in (8, 10, 12, 14):
                    base = 2 * (mc - 8)
                    for kh in range(base, base + 4):
                        wv = wcpool.tile([P, NF], MM_DT, tag="wc", name=f"wv0_{kh}")
                        nc.scalar.dma_start(out=wv, in_=wout.ap()[kh, :, :NF])
                        wv0.append(wv)
                for sg in range(S_OWN // NF):
                    t0 = sg * NF
                    ps = pspool.tile([P, NF], F32, tag="ps")
                    for q in range(KQ):
                        nc.tensor.matmul(
                            out=ps,
                            lhsT=wts[0][:, q],
                            rhs=xT8_sb[:, 2 * q : 2 * q + 2, WARM + t0 : WARM + t0 + NF],
                            start=(q == 0),
                            stop=False,
                            perf_mode=DR,
                        )
                    for q in range(KQ):
                        nc.tensor.matmul(
                            out=ps,
                            lhsT=wts[1][:, q],
                            rhs=c8_sb[:, 2 * q : 2 * q + 2, t0 : t0 + NF],
                            start=False,
                            stop=(q == KQ - 1),
                            perf_mode=DR,
                        )
                    og = spool.tile([P, NF], MM_DT, tag="og")
                    nc.scalar.activation(
                        out=og,
                        in_=ps,
                        func=act.Sigmoid,
                        bias=bo_sb[:, mc : mc + 1],
                        scale=1.0 / WSCALE,
                    )
                    nc.vector.tensor_mul(
                        out=mog_sb[:, mc, t0 : t0 + NF],
                        in0=og,
                        in1=c_sb[:, mc, WARM + t0 : WARM + t0 + NF],
                    )

            # ---- Stage C: o = (og*c) @ Wout + bout, natural [time, d_out]
            for n in range(D_OUT // NF):
                if n == 0:
                    wvs = wv0
                else:
                    wvs = []
                    for kh in range(MC):
                        wv = wcpool.tile([P, NF], MM_DT, tag="wc")
                        nc.scalar.dma_start(
                            out=wv, in_=wout.ap()[kh, :, NF * n : NF * (n + 1)]
                        )
                        wvs.append(wv)
                for sc in range(S_OWN // P):
                    ps = pspool.tile([P, NF], F32, tag="ps")
                    for kh in range(MC):
                        nc.tensor.matmul(
                            out=ps,
                            lhsT=mog_sb[:, kh, P * sc : P * (sc + 1)],
                            rhs=wvs[kh],
                            start=(kh == 0),
                            stop=(kh == MC - 1),
                        )
                    o_sb = spool.tile([P, NF], F32, tag="osb")
                    nc.vector.tensor_add(
                        out=o_sb, in0=ps, in1=bout_sb[:, NF * n : NF * (n + 1)]
                    )
                    for h in range(2):
                        nc.gpsimd.dma_start(
                            out=out.ap()[
                                P * sc + 64 * h : P * sc + 64 * (h + 1),
                                NF * n : NF * (n + 1),
                            ],
                            in_=o_sb[64 * h : 64 * (h + 1)],
                        )

    nc.compile()
    return nc


def get_module():
    if "nc" not in _BUILD_CACHE:
        _BUILD_CACHE["nc"] = build_module()
    return _BUILD_CACHE["nc"]


def _tile_wa(W):
    """[D_IN, D_HID] fp32 -> [MC, P, KC*P] fp16 stationary tiles (4KB lines)."""
    W = np.asarray(W, np.float32).astype(MM_NP)
    return np.ascontiguousarray(
        W.reshape(KC, P, MC, P).transpose(2, 1, 0, 3).reshape(MC, P, KC * P)
    )


def _tile_w8(W):
    """[D_IN, D_HID] fp32 -> [MC, P, KQ*2*P] fp8 DoubleRow-interleaved tiles."""
    import ml_dtypes

    W = np.asarray(W, np.float32) * WSCALE
    W8 = np.clip(W, -240, 240).astype(ml_dtypes.float8_e4m3)
    return np.ascontiguousarray(
        W8.reshape(KQ, 2, P, MC, P).transpose(3, 2, 0, 1, 4).reshape(MC, P, KQ * 2 * P)
    )


def _bias_t(b):
    """[D_HID] -> [P, MC] with partition-major layout."""
    return np.ascontiguousarray(np.asarray(b, np.float32).reshape(MC, P).T)


def prepare_in_maps(x, Wf, bf, Wi, bi, Wh, bh, Wo, bo, Wout, bout):
    import ml_dtypes

    x = np.asarray(x, np.float32)
    Wo = np.asarray(Wo, np.float32)

    xT_pad = np.zeros((D_IN, WARM + SEQ), MM_NP)
    xT_pad[:, WARM:] = x.T.astype(MM_NP)
    xT8_pad = np.zeros((D_IN, WARM + SEQ), ml_dtypes.float8_e4m3)
    xT8_pad[:, WARM:] = np.clip(x.T, -240, 240).astype(ml_dtypes.float8_e4m3)

    wg_host = np.stack([_tile_wa(Wi), _tile_wa(Wh)])
    wf8_host = _tile_w8(Wf)
    wo_host = np.stack([_tile_w8(Wo[:D_IN]), _tile_w8(Wo[D_IN:])])
    wout_host = np.ascontiguousarray(
        np.asarray(Wout, np.float32).astype(MM_NP).reshape(MC, P, D_OUT)
    )
    bg_host = np.ascontiguousarray(
        np.stack([_bias_t(bf), _bias_t(bi), _bias_t(bh)], axis=1)
    )
    bo_host = _bias_t(bo)
    bout_host = np.ascontiguousarray(
        np.broadcast_to(np.asarray(bout, np.float32), (P, D_OUT))
    )

    shared = {
        "Wg": wg_host,
        "Wf8": wf8_host,
        "Wo8": wo_host,
        "Wout": wout_host,
        "bg": bg_host,
        "bo": bo_host,
        "bout": bout_host,
    }
    in_maps = []
    for k in range(N_CORES):
        xk = np.ascontiguousarray(xT_pad[:, k * S_OWN : k * S_OWN + S_TOT])
        xk8 = np.ascontiguousarray(xT8_pad[:, k * S_OWN : k * S_OWN + S_TOT])
        in_maps.append({"xT": xk, "xT8": xk8, **shared})
    return in_maps


def kernel(x, Wf, bf, Wi, bi, Wh, bh, Wo, bo, Wout, bout, _trace=False):
    in_maps = prepare_in_maps(x, Wf, bf, Wi, bi, Wh, bh, Wo, bo, Wout, bout)
    nc = get_module()
    res = run_bass_kernel_spmd(nc, in_maps, core_ids=list(range(N_CORES)), trace=_trace)
    _BUILD_CACHE["last_result"] = res
    return np.concatenate([r["out"] for r in res.results], axis=0)


# revision 22
# speedup vs baseline: 1.0597x; 1.0597x over previous
"""Trainium2 Bass kernel for the HPLSTM module (8-core SPMD, sequence-parallel).

Math (per reference):
    fg = sigmoid(x @ Wf + bf)
    hr = sigmoid(x @ Wi + bi) * tanh(x @ Wh + bh)
    c_t = fg_t * c_{t-1} + hr_t              (linear scan over time)
    og = sigmoid([x, c] @ Wo + bo)
    o  = (og * c) @ Wout + bout

Sharding: sequence-parallel. Core k owns timesteps [k*1024, (k+1)*1024) and
additionally recomputes a WARM-step prefix to derive its scan initial
condition locally (forget gates are sigmoid(~N(0,1)), so carry contributions
decay like ~0.45^t and are far below fp32 resolution after WARM steps). No
cross-core communication at all.

Layout: activations live transposed as [hidden, time] so the recurrence runs
along the SBUF free axis via the DVE tensor_tensor_scan instruction. The
final projection consumes (og*c) in [hidden, time] layout directly as the
matmul stationary operand and produces output in natural [time, d_out]
orientation, so no transpose-back is needed.

Precision: the i/h gate GEMMs and the output projection run in fp16 (fp32
PSUM accumulation). The forget-gate GEMM and the output-gate GEMM run in
fp8-e4m3 with perf_mode=DoubleRow (2 fp8 weights/PE cell, 256-deep
contraction per matmul, ~2x PE throughput) — these are the two spots where
fp8 quantization noise barely reaches the output (sigmoid gain 0.25; the
scan and the og path attenuate further). Wf/Wo are pre-scaled by 64
host-side so their entries sit in e4m3's normal range; the sigmoid
activations un-scale via their scale argument.

Loop order: contraction-outer, n-chunk-inner over multiple live PSUM banks,
so every LDWEIGHTS hides under a preceding 512-wide matmul (a 64-wide
matmul can't cover a weight load on its own).

DMA: all HBM loads use >=2KB per-partition lines (the per-queue descriptor
feed is ~78ns/line — smaller lines are feed-bound) and big tensors are
split across several dma_starts so the round-robin queue assignment spreads
them over the 16 HW queues.
"""

import numpy as np

import concourse.bacc as bacc
import concourse.mybir as mybir
import concourse.tile as tile
from concourse.bass_utils import run_bass_kernel_spmd

SEQ, D_IN, D_HID, D_OUT = 8192, 2048, 2048, 2048
N_CORES = 8
P = 128
S_OWN = SEQ // N_CORES          # 1024 timesteps owned per core
WARM = 64                       # truncated-carry warmup prefix
S_TOT = S_OWN + WARM            # 1088 time columns held per core
KC = D_IN // P                  # 16 contraction chunks over d_in
MC = D_HID // P                 # 16 chunks over hidden
KQ = KC // 2                    # 8 DoubleRow chunks (256-deep) over d_in
NF = 512                        # PSUM moving free-dim

MM_DT = mybir.dt.float16        # fp16 matmul operand dtype (fp32 PSUM accum)
MM_NP = np.float16
F8 = mybir.dt.float8e4          # TRN e4m3 (max 240)
WSCALE = 64.0                   # host-side scale on Wf/Wo before fp8 cast

F32 = mybir.dt.float32

_BUILD_CACHE = {}


def build_module():
    """Build + compile the single-core BIR module (same NEFF on all 8 cores)."""
    act = mybir.ActivationFunctionType
    alu = mybir.AluOpType
    DR = mybir.MatmulPerfMode.DoubleRow

    nc = bacc.Bacc("TRN2", debug=False, num_devices=N_CORES)

    xT = nc.declare_dram_parameter("xT", [D_IN, S_TOT], MM_DT, isOutput=False)
    xT8 = nc.declare_dram_parameter("xT8", [D_IN, S_TOT], F8, isOutput=False)
    # i/h gate weights pre-tiled host-side: [2, MC, P, KC*P] (4KB lines)
    wg = nc.declare_dram_parameter("Wg", [2, MC, P, KC * P], MM_DT, isOutput=False)
    # forget-gate weights, fp8 DoubleRow-interleaved, pre-scaled by WSCALE:
    # [MC, P, KQ*2*P] with k = 256*q + 128*j + p
    wf8 = nc.declare_dram_parameter("Wf8", [MC, P, KQ * 2 * P], F8, isOutput=False)
    # output-gate weights (x-part, c-part), same fp8 layout
    wo = nc.declare_dram_parameter("Wo8", [2, MC, P, KQ * 2 * P], F8, isOutput=False)
    wout = nc.declare_dram_parameter("Wout", [MC, P, D_OUT], MM_DT, isOutput=False)
    bg = nc.declare_dram_parameter("bg", [P, 3, MC], F32, isOutput=False)
    bo = nc.declare_dram_parameter("bo", [P, MC], F32, isOutput=False)
    bout = nc.declare_dram_parameter("bout", [P, D_OUT], F32, isOutput=False)
    out = nc.declare_dram_parameter("out", [S_OWN, D_OUT], F32, isOutput=True)

    # n-chunks over the full (warm + owned) time range for the gate GEMMs
    NCH = [(0, NF), (NF, NF), (2 * NF, S_TOT - 2 * NF)]

    with tile.TileContext(nc) as tc:
        with (
            tc.tile_pool(name="singles", bufs=1) as singles,
            tc.tile_pool(name="wpool", bufs=4) as wpool,
            tc.tile_pool(name="w8pool", bufs=4) as w8pool,
            tc.tile_pool(name="wcpool", bufs=20) as wcpool,
            tc.tile_pool(name="gpool", bufs=2) as gpool,
            tc.tile_pool(name="spool", bufs=2) as spool,
            tc.tile_pool(name="psum", bufs=6, space="PSUM") as pspool,
        ):
            xT_sb = singles.tile([P, KC, S_TOT], MM_DT)
            xT8_sb = singles.tile([P, KC, S_TOT], F8)
            c_sb = singles.tile([P, MC, S_TOT], MM_DT)
            c8_sb = singles.tile([P, MC, S_OWN], F8)
            mog_sb = singles.tile([P, MC, S_OWN], MM_DT)
            bg_sb = singles.tile([P, 3, MC], F32)
            bo_sb = singles.tile([P, MC], F32)
            bout_sb = singles.tile([P, D_OUT], F32)

            def load_wa(g, mc):
                """i/h fp16 weight tile [P, KC*P]; 2KB lines, 2 calls."""
                wt = wpool.tile([P, KC * P], MM_DT, tag="w")
                src = wg.ap()[g, mc]
                for v in range(2):
                    nc.sync.dma_start(
                        out=wt[:, 1024 * v : 1024 * (v + 1)],
                        in_=src[:, 1024 * v : 1024 * (v + 1)],
                    )
                return wt

            def load_w8(src_ap, eng=None):
                """fp8 DoubleRow weight tile [P, KQ, 2, P]; 2KB lines, 2 calls."""
                wt = w8pool.tile([P, KQ, 2, P], F8, tag="w8")
                src = src_ap.rearrange("p (q j m) -> p q j m", q=KQ, j=2)
                e = eng if eng is not None else nc.sync
                e.dma_start(out=wt[:64], in_=src[:64])
                e.dma_start(out=wt[64:], in_=src[64:])
                return wt

            # ---- prologue DMAs. Each dma_start costs ~645ns of serial
            # dispatch on its engine's queue, so the critical path is call
            # COUNT on the sync queue: xT16 rows start dispatching
            # immediately (interleaved with the mc=0 weight calls), while
            # the fp8 operands (needed ~25us in) dispatch in parallel from
            # the otherwise-idle gpsimd queue and the biases from scalar.
            xT_t = xT.ap().rearrange("(kc p) t -> kc p t", p=P)
            xT8_t = xT8.ap().rearrange("(kc p) t -> kc p t", p=P)
            for kc in range(2):
                nc.sync.dma_start(out=xT_sb[:, kc], in_=xT_t[kc])
            wi0 = load_wa(0, 0)
            for kc in range(2, 8):
                nc.sync.dma_start(out=xT_sb[:, kc], in_=xT_t[kc])
            wh0 = load_wa(1, 0)
            for kc in range(8, KC):
                nc.sync.dma_start(out=xT_sb[:, kc], in_=xT_t[kc])
            wih_next = [wi0, wh0]
            nc.scalar.dma_start(out=bg_sb, in_=bg.ap())
            nc.scalar.dma_start(out=bo_sb, in_=bo.ap())
            wf_next = load_w8(wf8.ap()[0])
            xT8_pk = xT8.ap().rearrange("(kc p) t -> p kc t", p=P)
            for kc in range(0, KC, 2):
                nc.sync.dma_start(
                    out=xT8_sb[:, kc : kc + 2], in_=xT8_pk[:, kc : kc + 2]
                )
            nc.scalar.dma_start(out=bout_sb, in_=bout.ap())

            # ---- Stage A: gate GEMMs + activations + scan, per hidden chunk
            for mc in range(MC):
                # depth-1 weight prefetch: issue mc+1's loads before mc's
                # compute so their DMAs dispatch a full gate-group early
                wih_cur, wf_cur = wih_next, wf_next
                if mc + 1 < MC:
                    wih_next = [load_wa(g, mc + 1) for g in range(2)]
                    wf_next = load_w8(wf8.ap()[mc + 1])

                # i (g=1) and h (g=2) gates in fp16, kc-outer over 3 banks
                ih_tiles = []
                for g in (1, 2):
                    wt = wih_cur[g - 1]
                    g_sb = gpool.tile([P, S_TOT], MM_DT, tag=f"g{g}")
                    pss = [pspool.tile([P, nw], F32, tag="ps", name=f"ps{j}") for j, (_, nw) in enumerate(NCH)]
                    for kc in range(KC):
                        for j, (n0, nw) in enumerate(NCH):
                            nc.tensor.matmul(
                                out=pss[j],
                                lhsT=wt[:, P * kc : P * (kc + 1)],
                                rhs=xT_sb[:, kc, n0 : n0 + nw],
                                start=(kc == 0),
                                stop=(kc == KC - 1),
                            )
                    fn = act.Tanh if g == 2 else act.Sigmoid
                    for j, (n0, nw) in enumerate(NCH):
                        nc.scalar.activation(
                            out=g_sb[:, n0 : n0 + nw],
                            in_=pss[j],
                            func=fn,
                            bias=bg_sb[:, g, mc : mc + 1],
                        )
                    ih_tiles.append(g_sb)

                # forget gate in fp8 DoubleRow, kq-outer over 3 banks
                wtf = wf_cur
                f_sb = gpool.tile([P, S_TOT], MM_DT, tag="g0")
                pss = [pspool.tile([P, nw], F32, tag="ps", name=f"ps{j}") for j, (_, nw) in enumerate(NCH)]
                for q in range(KQ):
                    for j, (n0, nw) in enumerate(NCH):
                        nc.tensor.matmul(
                            out=pss[j],
                            lhsT=wtf[:, q],
                            rhs=xT8_sb[:, 2 * q : 2 * q + 2, n0 : n0 + nw],
                            start=(q == 0),
                            stop=(q == KQ - 1),
                            perf_mode=DR,
                        )
                for j, (n0, nw) in enumerate(NCH):
                    nc.scalar.activation(
                        out=f_sb[:, n0 : n0 + nw],
                        in_=pss[j],
                        func=act.Sigmoid,
                        bias=bg_sb[:, 0, mc : mc + 1],
                        scale=1.0 / WSCALE,
                    )

                # hr = sig(i)*tanh(h), in-place over the tanh tile
                nc.vector.tensor_mul(out=ih_tiles[1], in0=ih_tiles[0], in1=ih_tiles[1])
                # c_t = fg_t * c_{t-1} + hr_t along the free (time) axis
                nc.vector.tensor_tensor_scan(
                    out=c_sb[:, mc, :],
                    data0=f_sb,
                    data1=ih_tiles[1],
                    initial=0.0,
                    op0=alu.mult,
                    op1=alu.add,
                )
                # fp8 copy of the owned region for the stage-B moving operand
                # (last chunks go on the faster scalar engine — they gate
                # stage B's first PSUM group at the A->B boundary)
                if mc >= 14:
                    nc.scalar.copy(out=c8_sb[:, mc], in_=c_sb[:, mc, WARM:])
                else:
                    nc.gpsimd.tensor_copy(out=c8_sb[:, mc], in_=c_sb[:, mc, WARM:])

            # ---- Stage B: output gate over [x; c] in fp8 DoubleRow,
            # then mog = og * c
            wv0 = []
            for mc in range(MC):
                wts = [load_w8(wo.ap()[part, mc]) for part in range(2)]
                # prefetch stage C's first weight block in waves on the
                # scalar queue so it never delays stage B's own loads
                if mc in (8, 10, 12, 14):
                    base = 2 * (mc - 8)
                    for kh in range(base, base + 4):
                        wv = wcpool.tile([P, NF], MM_DT, tag="wc", name=f"wv0_{kh}")
                        nc.sync.dma_start(out=wv, in_=wout.ap()[kh, :, :NF])
                        wv0.append(wv)
                for sg in range(S_OWN // NF):
                    t0 = sg * NF
                    ps = pspool.tile([P, NF], F32, tag="ps")
                    for q in range(KQ):
                        nc.tensor.matmul(
                            out=ps,
                            lhsT=wts[0][:, q],
                            rhs=xT8_sb[:, 2 * q : 2 * q + 2, WARM + t0 : WARM + t0 + NF],
                            start=(q == 0),
                            stop=False,
                            perf_mode=DR,
                        )
                    for q in range(KQ):
                        nc.tensor.matmul(
                            out=ps,
                            lhsT=wts[1][:, q],
                            rhs=c8_sb[:, 2 * q : 2 * q + 2, t0 : t0 + NF],
                            start=False,
                            stop=(q == KQ - 1),
                            perf_mode=DR,
                        )
                    og = spool.tile([P, NF], MM_DT, tag="og")
                    nc.scalar.activation(
                        out=og,
                        in_=ps,
                        func=act.Sigmoid,
                        bias=bo_sb[:, mc : mc + 1],
                        scale=1.0 / WSCALE,
                    )
                    nc.vector.tensor_mul(
                        out=mog_sb[:, mc, t0 : t0 + NF],
                        in0=og,
                        in1=c_sb[:, mc, WARM + t0 : WARM + t0 + NF],
                    )

            # ---- Stage C: o = (og*c) @ Wout + bout, natural [time, d_out]
            for n in range(D_OUT // NF):
                if n == 0:
                    wvs = wv0
                else:
                    wvs = []
                    for kh in range(MC):
                        wv = wcpool.tile([P, NF], MM_DT, tag="wc")
                        nc.sync.dma_start(
                            out=wv, in_=wout.ap()[kh, :, NF * n : NF * (n + 1)]
                        )
                        wvs.append(wv)
                for sc in range(S_OWN // P):
                    ps = pspool.tile([P, NF], F32, tag="ps")
                    for kh in range(MC):
                        nc.tensor.matmul(
                            out=ps,
                            lhsT=mog_sb[:, kh, P * sc : P * (sc + 1)],
                            rhs=wvs[kh],
                            start=(kh == 0),
                            stop=(kh == MC - 1),
                        )
                    o_sb = spool.tile([P, NF], F32, tag="osb")
                    nc.vector.tensor_add(
                        out=o_sb, in0=ps, in1=bout_sb[:, NF * n : NF * (n + 1)]
                    )
                    for h in range(2):
                        nc.sync.dma_start(
                            out=out.ap()[
                                P * sc + 64 * h : P * sc + 64 * (h + 1),
                                NF * n : NF * (n + 1),
                            ],
                            in_=o_sb[64 * h : 64 * (h + 1)],
                        )

    nc.compile()
    return nc


def get_module():
    if "nc" not in _BUILD_CACHE:
        _BUILD_CACHE["nc"] = build_module()
    return _BUILD_CACHE["nc"]


def _tile_wa(W):
    """[D_IN, D_HID] fp32 -> [MC, P, KC*P] fp16 stationary tiles (4KB lines)."""
    W = np.asarray(W, np.float32).astype(MM_NP)
    return np.ascontiguousarray(
        W.reshape(KC, P, MC, P).transpose(2, 1, 0, 3).reshape(MC, P, KC * P)
    )


def _tile_w8(W):
    """[D_IN, D_HID] fp32 -> [MC, P, KQ*2*P] fp8 DoubleRow-interleaved tiles."""
    import ml_dtypes

    W = np.asarray(W, np.float32) * WSCALE
    W8 = np.clip(W, -240, 240).astype(ml_dtypes.float8_e4m3)
    return np.ascontiguousarray(
        W8.reshape(KQ, 2, P, MC, P).transpose(3, 2, 0, 1, 4).reshape(MC, P, KQ * 2 * P)
    )


def _bias_t(b):
    """[D_HID] -> [P, MC] with partition-major layout."""
    return np.ascontiguousarray(np.asarray(b, np.float32).reshape(MC, P).T)


def prepare_in_maps(x, Wf, bf, Wi, bi, Wh, bh, Wo, bo, Wout, bout):
    import ml_dtypes

    x = np.asarray(x, np.float32)
    Wo = np.asarray(Wo, np.float32)

    xT_pad = np.zeros((D_IN, WARM + SEQ), MM_NP)
    xT_pad[:, WARM:] = x.T.astype(MM_NP)
    xT8_pad = np.zeros((D_IN, WARM + SEQ), ml_dtypes.float8_e4m3)
    xT8_pad[:, WARM:] = np.clip(x.T, -240, 240).astype(ml_dtypes.float8_e4m3)

    wg_host = np.stack([_tile_wa(Wi), _tile_wa(Wh)])
    wf8_host = _tile_w8(Wf)
    wo_host = np.stack([_tile_w8(Wo[:D_IN]), _tile_w8(Wo[D_IN:])])
    wout_host = np.ascontiguousarray(
        np.asarray(Wout, np.float32).astype(MM_NP).reshape(MC, P, D_OUT)
    )
    bg_host = np.ascontiguousarray(
        np.stack([_bias_t(bf), _bias_t(bi), _bias_t(bh)], axis=1)
    )
    bo_host = _bias_t(bo)
    bout_host = np.ascontiguousarray(
        np.broadcast_to(np.asarray(bout, np.float32), (P, D_OUT))
    )

    shared = {
        "Wg": wg_host,
        "Wf8": wf8_host,
        "Wo8": wo_host,
        "Wout": wout_host,
        "bg": bg_host,
        "bo": bo_host,
        "bout": bout_host,
    }
    in_maps = []
    for k in range(N_CORES):
        xk = np.ascontiguousarray(xT_pad[:, k * S_OWN : k * S_OWN + S_TOT])
        xk8 = np.ascontiguousarray(xT8_pad[:, k * S_OWN : k * S_OWN + S_TOT])
        in_maps.append({"xT": xk, "xT8": xk8, **shared})
    return in_maps


def kernel(x, Wf, bf, Wi, bi, Wh, bh, Wo, bo, Wout, bout, _trace=False):
    in_maps = prepare_in_maps(x, Wf, bf, Wi, bi, Wh, bh, Wo, bo, Wout, bout)
    nc = get_module()
    res = run_bass_kernel_spmd(nc, in_maps, core_ids=list(range(N_CORES)), trace=_trace)
    _BUILD_CACHE["last_result"] = res
    return np.concatenate([r["out"] for r in res.results], axis=0)
